# revision 11
# baseline (speedup 1.0000x reference)
"""Trainium2 Bass kernel for nn_Conv2d_14147622273082.

Conv2d 3x3, stride 1, pad 1: x [8, 320, 64, 64] f32, hf8-coded weights
w_bits [320, 320, 3, 3] i32 (codes 0..255), bias codes b_bits [320] i32.
out = conv2d(x, hf8_decode(w_bits)) + hf8_decode(b_bits).

Strategy: data-parallel over batch (1 image per NeuronCore, 8 cores).
Weights replicated; hf8 decode on-device via a bit trick:
hf8(1-4-3, bias 14) == bitcast_f32(sign<<31 | code7<<20) * 2^113
(exact, incl. subnormals). The conv is 9 shifted [Cin,Cout] x [Cin,pix]
fp16 matmuls accumulated in PSUM over a zero-padded fp16 input image.

Cin=320 splits into K-chunks (128, 128, 64). The 64-wide tail would waste
half the PE array, so kernel positions are packed in pairs: partitions
0:64 hold the tail channels, partitions 64:128 hold the same channels
with the padded image pre-shifted by the delta between the two positions
(flat +1 == next column; flat +66 == (row+1, col-2) in the 68-wide pad),
so one K=128 matmul computes two positions at once. 9 positions -> 4
pairs + 1 solo: 23 instead of 27 accumulating matmuls per PSUM tile.
"""

import numpy as np

import concourse.bass as bass
import concourse.tile as tile
from concourse import bacc, mybir
from concourse.bass_utils import run_bass_kernel_spmd

B, CIN, COUT, H, W = 8, 320, 320, 64, 64
PIX = H * W  # 4096
P = 128
CO_CHUNKS = [(0, 128), (128, 256), (256, 320)]
N_TILE = 512  # pixels per psum tile = 8 rows of 64
ROWS_PER_TILE = N_TILE // W  # 8
N_PIX_TILES = PIX // N_TILE  # 8
# padded image: rows 0..65 (top/bottom zero), cols: 2 left / 2 right zero
HP, WP = H + 2, W + 4  # 66 x 68 (even left pad keeps fp16 writes 4B-aligned)

# tail position pairing: (pos_a, pos_b) packed on partitions (0:64, 64:128).
# delta = flat_offset(b) - flat_offset(a) in the padded [66,68] layout.
# pairs with delta 1 share the "+1 shifted" upper image (xp2 upper half);
# the delta-66 pair gets its own tile (XB).
TAIL_PAIRS = [(0, 1), (2, 3), (4, 5), (6, 7)]  # pos = kh*3+kw
TAIL_SOLO = 8

F16 = mybir.dt.float16
F32 = mybir.dt.float32
I32 = mybir.dt.int32
HF8_SCALE = float(2.0**113)


def _decode_hf8(nc, pool, codes_ap, out_ap, nparts, free, tag):
    """out = hf8_decode(codes) = bitcast_f32(sign<<31 | code7<<20) * 2^113."""
    t1 = pool.tile([P, free], I32, tag=f"{tag}_t1", name=f"{tag}_t1")
    t2 = pool.tile([P, free], I32, tag=f"{tag}_t2", name=f"{tag}_t2")
    nc.vector.tensor_scalar(
        t1[:nparts], codes_ap, 0x80, 24,
        mybir.AluOpType.bitwise_and, mybir.AluOpType.logical_shift_left,
    )
    nc.vector.tensor_scalar(
        t2[:nparts], codes_ap, 0x7F, 20,
        mybir.AluOpType.bitwise_and, mybir.AluOpType.logical_shift_left,
    )
    nc.vector.tensor_tensor(
        t1[:nparts], t1[:nparts], t2[:nparts], mybir.AluOpType.bitwise_or
    )
    nc.vector.tensor_scalar_mul(out_ap, t1[:nparts].bitcast(F32), HF8_SCALE)


def _pad_borders(nc, xt, col_lo, col_hi, parts=slice(0, P), rows=(0, HP - 1)):
    """Zero the pad borders around an interior written at cols [col_lo, col_hi)."""
    nc.vector.memset(xt[parts, rows[0] : rows[0] + 1, :], 0.0)
    nc.vector.memset(xt[parts, rows[1] : rows[1] + 1, :], 0.0)
    if col_lo > 0:
        nc.vector.memset(xt[parts, rows[0] + 1 : rows[1], 0:col_lo], 0.0)
    if col_hi < WP:
        nc.vector.memset(xt[parts, rows[0] + 1 : rows[1], col_hi:WP], 0.0)


def build():
    nc = bacc.Bacc(
        "TRN2", target_bir_lowering=False, debug=False, enable_partition_id=False
    )
    x_d = nc.dram_tensor("x", [CIN, PIX], F32, kind="ExternalInput")
    w_d = nc.dram_tensor("w9", [CIN, 9, COUT], I32, kind="ExternalInput")
    b_d = nc.dram_tensor("b", [COUT, 1], I32, kind="ExternalInput")
    out_d = nc.dram_tensor("out", [COUT, PIX], F32, kind="ExternalOutput")

    with tile.TileContext(nc) as tc:
        with (
            tc.tile_pool(name="persist", bufs=1) as persist,
            tc.tile_pool(name="stage", bufs=2) as stage,
            tc.tile_pool(name="outsb", bufs=4) as outsb,
            tc.tile_pool(name="psum", bufs=1, space="PSUM") as psum_pool,
        ):
            wl = [None, None]  # full ci chunks 0:128, 128:256 -> [P, 9, CO] f16
            xp = [None, None]  # padded images for those chunks
            wpair = None  # tail weights [P, 5, CO] f16 (paired positions)
            xp2 = None  # tail image: lower normal, upper shifted +1
            xb2 = None  # tail image: lower normal, upper shifted +66
            bias = []

            # ---- chunk 0 / 1 (full 128-channel chunks), interleaved ----
            # chunk 0 is on the critical path to the first matmul: its weight
            # DMA + decode and image DMA + cast are split in halves so decode
            # and cast start as soon as the first half lands. w/b DMAs ride
            # the sync queue, x DMAs the gpsimd queue (parallel rings); pad
            # casts run on the Scalar engine to keep DVE free for decode.
            for ci in range(2):
                cs, ce = ci * P, (ci + 1) * P
                wraw = stage.tile([P, 9, COUT], I32, tag="wraw", name="wraw")
                wt = persist.tile([P, 9, COUT], F16, tag=f"wl{ci}", name=f"wl{ci}")
                xs = stage.tile([P, H, W], F32, tag="xstage", name="xstage")
                xt = persist.tile([P, HP, WP], F16, tag=f"xpad{ci}", name=f"xpad{ci}")
                _pad_borders(nc, xt, 2, W + 2)
                wflat = wraw.rearrange("p a b -> p (a b)")
                oflat = wt.rearrange("p a b -> p (a b)")
                if ci == 0:
                    half = 5 * COUT
                    nc.sync.dma_start(wflat[:, :half], w_d[cs:ce, :5])
                    nc.gpsimd.dma_start(
                        xs[:, : H // 2],
                        x_d[cs:ce, : PIX // 2].rearrange("p (h w) -> p h w", h=H // 2),
                    )
                    nc.sync.dma_start(wflat[:, half:], w_d[cs:ce, 5:])
                    nc.gpsimd.dma_start(
                        xs[:, H // 2 :],
                        x_d[cs:ce, PIX // 2 :].rearrange("p (h w) -> p h w", h=H // 2),
                    )
                    _decode_hf8(nc, stage, wflat[:, :half], oflat[:, :half],
                                P, half, "wdec")
                    nc.scalar.copy(
                        xt[:, 1 : H // 2 + 1, 2 : W + 2], xs[:, : H // 2]
                    )
                    _decode_hf8(nc, stage, wflat[:, half:], oflat[:, half:],
                                P, 9 * COUT - half, "wdec")
                    nc.scalar.copy(
                        xt[:, H // 2 + 1 : H + 1, 2 : W + 2], xs[:, H // 2 :]
                    )
                else:
                    nc.sync.dma_start(wraw[:], w_d[cs:ce])
                    nc.gpsimd.dma_start(
                        xs[:], x_d[cs:ce].rearrange("p (h w) -> p h w", h=H)
                    )
                    _decode_hf8(nc, stage, wflat, oflat, P, 9 * COUT, "wdec")
                    nc.scalar.copy(xt[:, 1 : H + 1, 2 : W + 2], xs[:])
                wl[ci] = wt
                xp[ci] = xt

            # ---- tail chunk (ci 256:320, 64 channels) with position pairing ----
            cs, ce = 256, 320
            wraw2 = stage.tile([P, 5, COUT], I32, tag="wraw2", name="wraw2")
            for j, (pa, pb) in enumerate(TAIL_PAIRS):
                nc.sync.dma_start(wraw2[0:64, j], w_d[cs:ce, pa])
                nc.sync.dma_start(wraw2[64:128, j], w_d[cs:ce, pb])
            nc.sync.dma_start(wraw2[0:64, 4], w_d[cs:ce, TAIL_SOLO])
            nc.vector.memset(wraw2[64:128, 4], 0)
            wpair = persist.tile([P, 5, COUT], F16, tag="wpair", name="wpair")
            _decode_hf8(
                nc, stage,
                wraw2.rearrange("p a b -> p (a b)"),
                wpair.rearrange("p a b -> p (a b)"),
                P, 5 * COUT, "wdec2",
            )

            # tail image staged twice (lower + upper partition halves)
            xs2 = stage.tile([P, H, W], F32, tag="xstage", name="xstage2")
            nc.gpsimd.dma_start(xs2[0:64], x_d[cs:ce].rearrange("p (h w) -> p h w", h=H))
            nc.gpsimd.dma_start(xs2[64:128], x_d[cs:ce].rearrange("p (h w) -> p h w", h=H))

            # Reading the upper half with pos_a's window offsets must yield
            # pos_b's window: place the upper interior at (1-dkh, 2-dkw).
            # xp2: lower = padded image; upper = interior at (1, 1) — serves
            # the (dkh,dkw)=(0,1) pairs.
            xp2 = persist.tile([P, HP, WP], F16, tag="xpad2", name="xpad2")
            _pad_borders(nc, xp2, 2, W + 2, parts=slice(0, 64))
            _pad_borders(nc, xp2, 1, W + 1, parts=slice(64, P))
            nc.scalar.copy(xp2[0:64, 1 : H + 1, 2 : W + 2], xs2[0:64])
            nc.scalar.copy(xp2[64:128, 1 : H + 1, 1 : W + 1], xs2[64:128])

            # xb2: lower = padded image; upper = interior at (0, 4) — serves
            # the (2,3) pair with (dkh,dkw)=(1,-2).
            xb2 = persist.tile([P, HP, WP], F16, tag="xpadb", name="xpadb")
            _pad_borders(nc, xb2, 2, W + 2, parts=slice(0, 64))
            nc.vector.memset(xb2[64:128, H : HP, :], 0.0)
            nc.vector.memset(xb2[64:128, 0:H, 0:4], 0.0)
            nc.scalar.copy(xb2[0:64, 1 : H + 1, 2 : W + 2], xs2[0:64])
            nc.scalar.copy(xb2[64:128, 0:H, 4:WP], xs2[64:128])

            # ---- bias: [320,1] i32 -> three [p,1] f32 tiles ----
            for mi, (ms, me) in enumerate(CO_CHUNKS):
                pm = me - ms
                braw = stage.tile([P, 1], I32, tag="braw", name="braw")
                nc.sync.dma_start(braw[:pm], b_d[ms:me, :])
                bf = persist.tile([P, 1], F32, tag=f"bias{mi}", name=f"bias{mi}")
                _decode_hf8(nc, stage, braw[:pm], bf[:pm], pm, 1, "bdec")
                bias.append(bf)

            # ---- matmuls: out[co, pix] += w[ci,co].T @ x_shift[ci, pix] ----
            n_acc = 2 * 9 + len(TAIL_PAIRS) + 1  # 23 per psum tile
            for mi, (ms, me) in enumerate(CO_CHUNKS):
                pm = me - ms
                acc = [
                    psum_pool.tile([P, N_TILE], F32, tag=f"acc{t}", name=f"acc_{mi}_{t}")
                    for t in range(N_PIX_TILES)
                ]
                k = 0
                for ci in range(2):
                    for kh in range(3):
                        for kw in range(3):
                            lhsT = wl[ci][:, kh * 3 + kw, ms:me]
                            for t in range(N_PIX_TILES):
                                h0 = t * ROWS_PER_TILE
                                rhs = xp[ci][
                                    :,
                                    h0 + kh : h0 + kh + ROWS_PER_TILE,
                                    kw + 1 : kw + 1 + W,
                                ]
                                nc.tensor.matmul(
                                    acc[t][:pm], lhsT, rhs,
                                    start=(k == 0), stop=(k == n_acc - 1),
                                )
                            k += 1
                # paired tail positions: K=128, upper half pre-shifted
                for j, (pa, pb) in enumerate(TAIL_PAIRS):
                    kh, kw = pa // 3, pa % 3
                    src = xb2 if (pa, pb) == (2, 3) else xp2
                    lhsT = wpair[:, j, ms:me]
                    for t in range(N_PIX_TILES):
                        h0 = t * ROWS_PER_TILE
                        rhs = src[
                            :, h0 + kh : h0 + kh + ROWS_PER_TILE, kw + 1 : kw + 1 + W
                        ]
                        nc.tensor.matmul(
                            acc[t][:pm], lhsT, rhs,
                            start=(k == 0), stop=(k == n_acc - 1),
                        )
                    k += 1
                # solo tail position (2,2): K=64
                kh, kw = 2, 2
                lhsT = wpair[0:64, 4, ms:me]
                for t in range(N_PIX_TILES):
                    h0 = t * ROWS_PER_TILE
                    rhs = xp2[0:64, h0 + kh : h0 + kh + ROWS_PER_TILE, kw + 1 : kw + 1 + W]
                    nc.tensor.matmul(
                        acc[t][:pm], lhsT, rhs,
                        start=(k == 0), stop=(k == n_acc - 1),
                    )
                k += 1
                assert k == n_acc

                for t in range(N_PIX_TILES):
                    osb = outsb.tile([P, N_TILE], F32, tag="osb", name="osb")
                    nc.scalar.activation(
                        osb[:pm], acc[t][:pm],
                        mybir.ActivationFunctionType.Identity,
                        bias=bias[mi][:pm], scale=1.0,
                    )
                    nc.sync.dma_start(
                        out_d[ms:me, t * N_TILE : (t + 1) * N_TILE], osb[:pm]
                    )

    nc.compile()
    return nc


_NC_CACHE = None


def _get_nc():
    global _NC_CACHE
    if _NC_CACHE is None:
        _NC_CACHE = build()
    return _NC_CACHE


def _prep_in_maps(x, w_bits, b_bits):
    # w_bits [co, ci, kh, kw] -> [ci, kh*3+kw, co] (host relayout only)
    w9 = np.ascontiguousarray(
        w_bits.astype(np.int32).transpose(1, 2, 3, 0).reshape(CIN, 9, COUT)
    )
    b2 = np.ascontiguousarray(b_bits.astype(np.int32).reshape(COUT, 1))
    return [
        {
            "x": np.ascontiguousarray(x[i].reshape(CIN, PIX).astype(np.float32)),
            "w9": w9,
            "b": b2,
        }
        for i in range(B)
    ]


def kernel(x, w_bits, b_bits):
    nc = _get_nc()
    in_maps = _prep_in_maps(x, w_bits, b_bits)
    res = run_bass_kernel_spmd(nc, in_maps, core_ids=list(range(B)), trace=False)
    return np.stack(
        [res.results[i]["out"].reshape(COUT, H, W) for i in range(B)]
    ).astype(np.float32)


if __name__ == "__main__":
    rng = np.random.default_rng(0)
    x = rng.standard_normal((B, CIN, H, W)).astype(np.float32)
    w_bits = rng.integers(0, 256, (COUT, CIN, 3, 3)).astype(np.int32)
    b_bits = rng.integers(0, 256, (COUT,)).astype(np.int32)
    out = kernel(x, w_bits, b_bits)
    print("out", out.shape, out.dtype, float(np.abs(out).mean()))


# revision 12
# speedup vs baseline: 1.0429x; 1.0429x over previous
"""Trainium2 Bass kernel for nn_Conv2d_14147622273082.

Conv2d 3x3, stride 1, pad 1: x [8, 320, 64, 64] f32, hf8-coded weights
w_bits [320, 320, 3, 3] i32 (codes 0..255), bias codes b_bits [320] i32.
out = conv2d(x, hf8_decode(w_bits)) + hf8_decode(b_bits).

Strategy: data-parallel over batch (1 image per NeuronCore, 8 cores).
Weights replicated; hf8 decode on-device via a bit trick:
hf8(1-4-3, bias 14) == bitcast_f32(sign<<31 | code7<<20) * 2^113
(exact, incl. subnormals). The conv is 9 shifted [Cin,Cout] x [Cin,pix]
fp16 matmuls accumulated in PSUM over a zero-padded fp16 input image.

Cin=320 splits into K-chunks (128, 128, 64). The 64-wide tail would waste
half the PE array, so kernel positions are packed in pairs: partitions
0:64 hold the tail channels, partitions 64:128 hold the same channels
with the padded image pre-shifted by the delta between the two positions
(flat +1 == next column; flat +66 == (row+1, col-2) in the 68-wide pad),
so one K=128 matmul computes two positions at once. 9 positions -> 4
pairs + 1 solo: 23 instead of 27 accumulating matmuls per PSUM tile.
"""

import numpy as np

import concourse.bass as bass
import concourse.tile as tile
from concourse import bacc, mybir
from concourse.bass_utils import run_bass_kernel_spmd

B, CIN, COUT, H, W = 8, 320, 320, 64, 64
PIX = H * W  # 4096
P = 128
CO_CHUNKS = [(0, 128), (128, 256), (256, 320)]
N_TILE = 512  # pixels per psum tile = 8 rows of 64
ROWS_PER_TILE = N_TILE // W  # 8
N_PIX_TILES = PIX // N_TILE  # 8
# padded image: rows 0..65 (top/bottom zero), cols: 2 left / 2 right zero
HP, WP = H + 2, W + 4  # 66 x 68 (even left pad keeps fp16 writes 4B-aligned)

# tail position pairing: (pos_a, pos_b) packed on partitions (0:64, 64:128).
# delta = flat_offset(b) - flat_offset(a) in the padded [66,68] layout.
# pairs with delta 1 share the "+1 shifted" upper image (xp2 upper half);
# the delta-66 pair gets its own tile (XB).
TAIL_PAIRS = [(0, 1), (2, 3), (4, 5), (6, 7)]  # pos = kh*3+kw
TAIL_SOLO = 8

F16 = mybir.dt.float16
F32 = mybir.dt.float32
I32 = mybir.dt.int32
HF8_SCALE = float(2.0**113)


def _decode_hf8(nc, pool, codes_ap, out_ap, nparts, free, tag):
    """out = hf8_decode(codes) = bitcast_f32(sign<<31 | code7<<20) * 2^113."""
    t1 = pool.tile([P, free], I32, tag=f"{tag}_t1", name=f"{tag}_t1")
    t2 = pool.tile([P, free], I32, tag=f"{tag}_t2", name=f"{tag}_t2")
    nc.vector.tensor_scalar(
        t1[:nparts], codes_ap, 0x80, 24,
        mybir.AluOpType.bitwise_and, mybir.AluOpType.logical_shift_left,
    )
    nc.vector.tensor_scalar(
        t2[:nparts], codes_ap, 0x7F, 20,
        mybir.AluOpType.bitwise_and, mybir.AluOpType.logical_shift_left,
    )
    nc.vector.tensor_tensor(
        t1[:nparts], t1[:nparts], t2[:nparts], mybir.AluOpType.bitwise_or
    )
    nc.vector.tensor_scalar_mul(out_ap, t1[:nparts].bitcast(F32), HF8_SCALE)


def _pad_borders(nc, xt, col_lo, col_hi, parts=slice(0, P), rows=(0, HP - 1)):
    """Zero the pad borders around an interior written at cols [col_lo, col_hi)."""
    nc.vector.memset(xt[parts, rows[0] : rows[0] + 1, :], 0.0)
    nc.vector.memset(xt[parts, rows[1] : rows[1] + 1, :], 0.0)
    if col_lo > 0:
        nc.vector.memset(xt[parts, rows[0] + 1 : rows[1], 0:col_lo], 0.0)
    if col_hi < WP:
        nc.vector.memset(xt[parts, rows[0] + 1 : rows[1], col_hi:WP], 0.0)


def build():
    nc = bacc.Bacc(
        "TRN2", target_bir_lowering=False, debug=False, enable_partition_id=False
    )
    x_d = nc.dram_tensor("x", [CIN, PIX], F32, kind="ExternalInput")
    w_d = nc.dram_tensor("w9", [CIN, 9, COUT], I32, kind="ExternalInput")
    b_d = nc.dram_tensor("b", [COUT, 1], I32, kind="ExternalInput")
    out_d = nc.dram_tensor("out", [COUT, PIX], F32, kind="ExternalOutput")

    with tile.TileContext(nc) as tc:
        with (
            tc.tile_pool(name="persist", bufs=1) as persist,
            tc.tile_pool(name="stage", bufs=2) as stage,
            tc.tile_pool(name="outsb", bufs=4) as outsb,
            tc.tile_pool(name="psum", bufs=1, space="PSUM") as psum_pool,
        ):
            wl = [None, None]  # full ci chunks 0:128, 128:256 -> [P, 9, CO] f16
            xp = [None, None]  # padded images for those chunks
            wpair = None  # tail weights [P, 5, CO] f16 (paired positions)
            xp2 = None  # tail image: lower normal, upper shifted +1
            xb2 = None  # tail image: lower normal, upper shifted +66
            bias = []

            # ---- chunk 0 / 1 (full 128-channel chunks), interleaved ----
            # chunk 0 is on the critical path to the first matmul: its weight
            # DMA + decode and image DMA + cast are split in halves so decode
            # and cast start as soon as the first half lands. w/b DMAs ride
            # the sync queue, x DMAs the gpsimd queue (parallel rings); pad
            # casts run on the Scalar engine to keep DVE free for decode.
            for ci in range(2):
                cs, ce = ci * P, (ci + 1) * P
                wraw = stage.tile([P, 9, COUT], I32, tag="wraw", name="wraw")
                wt = persist.tile([P, 9, COUT], F16, tag=f"wl{ci}", name=f"wl{ci}")
                xs = stage.tile([P, H, W], F32, tag="xstage", name="xstage")
                xt = persist.tile([P, HP, WP], F16, tag=f"xpad{ci}", name=f"xpad{ci}")
                _pad_borders(nc, xt, 2, W + 2)
                wflat = wraw.rearrange("p a b -> p (a b)")
                oflat = wt.rearrange("p a b -> p (a b)")
                if ci == 0:
                    half = 5 * COUT
                    nc.sync.dma_start(wflat[:, :half], w_d[cs:ce, :5])
                    nc.gpsimd.dma_start(
                        xs[:, : H // 2],
                        x_d[cs:ce, : PIX // 2].rearrange("p (h w) -> p h w", h=H // 2),
                    )
                    nc.sync.dma_start(wflat[:, half:], w_d[cs:ce, 5:])
                    nc.gpsimd.dma_start(
                        xs[:, H // 2 :],
                        x_d[cs:ce, PIX // 2 :].rearrange("p (h w) -> p h w", h=H // 2),
                    )
                    _decode_hf8(nc, stage, wflat[:, :half], oflat[:, :half],
                                P, half, "wdec")
                    nc.vector.tensor_copy(
                        xt[:, 1 : H // 2 + 1, 2 : W + 2], xs[:, : H // 2]
                    )
                    nc.vector.tensor_copy(
                        xt[:, H // 2 + 1 : H + 1, 2 : W + 2], xs[:, H // 2 :]
                    )
                    _decode_hf8(nc, stage, wflat[:, half:], oflat[:, half:],
                                P, 9 * COUT - half, "wdec")
                else:
                    nc.sync.dma_start(wraw[:], w_d[cs:ce])
                    nc.gpsimd.dma_start(
                        xs[:], x_d[cs:ce].rearrange("p (h w) -> p h w", h=H)
                    )
                    _decode_hf8(nc, stage, wflat, oflat, P, 9 * COUT, "wdec")
                    nc.vector.tensor_copy(xt[:, 1 : H + 1, 2 : W + 2], xs[:])
                wl[ci] = wt
                xp[ci] = xt

            # ---- tail chunk (ci 256:320, 64 channels) with position pairing ----
            cs, ce = 256, 320
            wraw2 = stage.tile([P, 5, COUT], I32, tag="wraw2", name="wraw2")
            for j, (pa, pb) in enumerate(TAIL_PAIRS):
                nc.sync.dma_start(wraw2[0:64, j], w_d[cs:ce, pa])
                nc.sync.dma_start(wraw2[64:128, j], w_d[cs:ce, pb])
            nc.sync.dma_start(wraw2[0:64, 4], w_d[cs:ce, TAIL_SOLO])
            nc.vector.memset(wraw2[64:128, 4], 0)
            wpair = persist.tile([P, 5, COUT], F16, tag="wpair", name="wpair")
            _decode_hf8(
                nc, stage,
                wraw2.rearrange("p a b -> p (a b)"),
                wpair.rearrange("p a b -> p (a b)"),
                P, 5 * COUT, "wdec2",
            )

            # tail image staged twice (lower + upper partition halves)
            xs2 = stage.tile([P, H, W], F32, tag="xstage", name="xstage2")
            nc.gpsimd.dma_start(xs2[0:64], x_d[cs:ce].rearrange("p (h w) -> p h w", h=H))
            nc.gpsimd.dma_start(xs2[64:128], x_d[cs:ce].rearrange("p (h w) -> p h w", h=H))

            # Reading the upper half with pos_a's window offsets must yield
            # pos_b's window: place the upper interior at (1-dkh, 2-dkw).
            # xp2: lower = padded image; upper = interior at (1, 1) — serves
            # the (dkh,dkw)=(0,1) pairs.
            xp2 = persist.tile([P, HP, WP], F16, tag="xpad2", name="xpad2")
            _pad_borders(nc, xp2, 2, W + 2, parts=slice(0, 64))
            _pad_borders(nc, xp2, 1, W + 1, parts=slice(64, P))
            nc.vector.tensor_copy(xp2[0:64, 1 : H + 1, 2 : W + 2], xs2[0:64])
            nc.vector.tensor_copy(xp2[64:128, 1 : H + 1, 1 : W + 1], xs2[64:128])

            # xb2: lower = padded image; upper = interior at (0, 4) — serves
            # the (2,3) pair with (dkh,dkw)=(1,-2).
            xb2 = persist.tile([P, HP, WP], F16, tag="xpadb", name="xpadb")
            _pad_borders(nc, xb2, 2, W + 2, parts=slice(0, 64))
            nc.vector.memset(xb2[64:128, H : HP, :], 0.0)
            nc.vector.memset(xb2[64:128, 0:H, 0:4], 0.0)
            nc.vector.tensor_copy(xb2[0:64, 1 : H + 1, 2 : W + 2], xs2[0:64])
            nc.vector.tensor_copy(xb2[64:128, 0:H, 4:WP], xs2[64:128])

            # ---- bias: [320,1] i32 -> three [p,1] f32 tiles ----
            for mi, (ms, me) in enumerate(CO_CHUNKS):
                pm = me - ms
                braw = stage.tile([P, 1], I32, tag="braw", name="braw")
                nc.sync.dma_start(braw[:pm], b_d[ms:me, :])
                bf = persist.tile([P, 1], F32, tag=f"bias{mi}", name=f"bias{mi}")
                _decode_hf8(nc, stage, braw[:pm], bf[:pm], pm, 1, "bdec")
                bias.append(bf)

            # ---- matmuls: out[co, pix] += w[ci,co].T @ x_shift[ci, pix] ----
            n_acc = 2 * 9 + len(TAIL_PAIRS) + 1  # 23 per psum tile
            for mi, (ms, me) in enumerate(CO_CHUNKS):
                pm = me - ms
                acc = [
                    psum_pool.tile([P, N_TILE], F32, tag=f"acc{t}", name=f"acc_{mi}_{t}")
                    for t in range(N_PIX_TILES)
                ]
                k = 0
                for ci in range(2):
                    for kh in range(3):
                        for kw in range(3):
                            lhsT = wl[ci][:, kh * 3 + kw, ms:me]
                            for t in range(N_PIX_TILES):
                                h0 = t * ROWS_PER_TILE
                                rhs = xp[ci][
                                    :,
                                    h0 + kh : h0 + kh + ROWS_PER_TILE,
                                    kw + 1 : kw + 1 + W,
                                ]
                                nc.tensor.matmul(
                                    acc[t][:pm], lhsT, rhs,
                                    start=(k == 0), stop=(k == n_acc - 1),
                                )
                            k += 1
                # paired tail positions: K=128, upper half pre-shifted
                for j, (pa, pb) in enumerate(TAIL_PAIRS):
                    kh, kw = pa // 3, pa % 3
                    src = xb2 if (pa, pb) == (2, 3) else xp2
                    lhsT = wpair[:, j, ms:me]
                    for t in range(N_PIX_TILES):
                        h0 = t * ROWS_PER_TILE
                        rhs = src[
                            :, h0 + kh : h0 + kh + ROWS_PER_TILE, kw + 1 : kw + 1 + W
                        ]
                        nc.tensor.matmul(
                            acc[t][:pm], lhsT, rhs,
                            start=(k == 0), stop=(k == n_acc - 1),
                        )
                    k += 1
                # solo tail position (2,2): K=64
                kh, kw = 2, 2
                lhsT = wpair[0:64, 4, ms:me]
                for t in range(N_PIX_TILES):
                    h0 = t * ROWS_PER_TILE
                    rhs = xp2[0:64, h0 + kh : h0 + kh + ROWS_PER_TILE, kw + 1 : kw + 1 + W]
                    nc.tensor.matmul(
                        acc[t][:pm], lhsT, rhs,
                        start=(k == 0), stop=(k == n_acc - 1),
                    )
                k += 1
                assert k == n_acc

                for t in range(N_PIX_TILES):
                    osb = outsb.tile([P, N_TILE], F32, tag="osb", name="osb")
                    nc.scalar.activation(
                        osb[:pm], acc[t][:pm],
                        mybir.ActivationFunctionType.Identity,
                        bias=bias[mi][:pm], scale=1.0,
                    )
                    nc.sync.dma_start(
                        out_d[ms:me, t * N_TILE : (t + 1) * N_TILE], osb[:pm]
                    )

    nc.compile()
    return nc


_NC_CACHE = None


def _get_nc():
    global _NC_CACHE
    if _NC_CACHE is None:
        _NC_CACHE = build()
    return _NC_CACHE


def _prep_in_maps(x, w_bits, b_bits):
    # w_bits [co, ci, kh, kw] -> [ci, kh*3+kw, co] (host relayout only)
    w9 = np.ascontiguousarray(
        w_bits.astype(np.int32).transpose(1, 2, 3, 0).reshape(CIN, 9, COUT)
    )
    b2 = np.ascontiguousarray(b_bits.astype(np.int32).reshape(COUT, 1))
    return [
        {
            "x": np.ascontiguousarray(x[i].reshape(CIN, PIX).astype(np.float32)),
            "w9": w9,
            "b": b2,
        }
        for i in range(B)
    ]


def kernel(x, w_bits, b_bits):
    nc = _get_nc()
    in_maps = _prep_in_maps(x, w_bits, b_bits)
    res = run_bass_kernel_spmd(nc, in_maps, core_ids=list(range(B)), trace=False)
    return np.stack(
        [res.results[i]["out"].reshape(COUT, H, W) for i in range(B)]
    ).astype(np.float32)


if __name__ == "__main__":
    rng = np.random.default_rng(0)
    x = rng.standard_normal((B, CIN, H, W)).astype(np.float32)
    w_bits = rng.integers(0, 256, (COUT, CIN, 3, 3)).astype(np.int32)
    b_bits = rng.integers(0, 256, (COUT,)).astype(np.int32)
    out = kernel(x, w_bits, b_bits)
    print("out", out.shape, out.dtype, float(np.abs(out).mean()))
